# revision 9
# baseline (speedup 1.0000x reference)
"""Distributed single-head attention block for one TRN2 chip (8 NeuronCores).

Math (per batch b):  Q = x@Wq.T, K = x@Wk.T, V = x@Wv.T,
                     out = softmax(Q K^T / sqrt(D)) V
Shapes: x [4, 4096, 256], W* [256, 256], out [4, 4096, 256] (f32).

Sharding: core c handles batch b = c//2, query half qc = c%2 (2048 queries),
with full K/V for that batch (K/V projection recomputed on both cores of a
batch pair -- it is tiny). All matmul inputs are pre-transposed & bf16-cast on
the host so that no on-chip transposes are needed.

Attention is permutation-invariant over keys, so each core receives x^T
ROTATED so that its own query half occupies columns [0:2048] -- Q projects
straight from the head of the same buffer K/V project from.

Schedule (v3):
  - input DMA split across 3 engine rings (sync/scalar/gpsimd) with the two
    head pieces sized to the minimum the first Q^T accumulation needs, and
    the x tail in 128-256KB chunks issued in consumption order.
  - scores are computed *transposed* (tiles [k=128, q=512]): PE matmul with
    lhsT = K^T tile, rhs = Q^T tile; exp runs on ScalarE straight out of
    PSUM (scale=1/16 folded in, no max subtraction: |scores| <= ~11).
  - attn^T tiles feed the AV matmul as the stationary operand with V [k, d]
    as moving; a ones column appended to V makes the same PSUM accumulation
    produce the softmax denominator.
  - the kernel returns the unnormalized numerator + denominator (D+1 f32
    columns); the final divide happens on the host. This removes all
    reciprocal/scale work from the device and shortens the tail: the PSUM
    accumulators are simply evicted f32 -> SBUF (DVE/ACT split) and DMAd.
  - the final q-tile drains its AV pipeline eagerly, splits the last exp
    into four 128-wide pieces (pipelining the last AVs behind them), and
    scatters its four sub-tiles over three engine rings.
"""

import os
import sys
from contextlib import ExitStack

sys.path.insert(0, "/opt/trn_rl_repo")

import numpy as np
import ml_dtypes

B, S, D = 4, 4096, 256
NCORES = 8
SQ = S // 2  # queries per core
P = 128  # SBUF partitions
EB = D // P  # e (contraction) blocks for projections
DB = D // P  # d blocks
KB = S // P  # key blocks of 128
QT = 512  # q tile (matmul moving free dim)
NQB = SQ // QT  # q tiles per core
SUBQ = QT // P  # 128-query sub-blocks per q tile
HC = 512  # head chunk: x columns packed with wq
NXCH = (S - HC) // QT  # 7 tail chunks of 512 columns

LAST_RESULT = None  # BassKernelResults of the most recent run (for test.py)
_CACHE = {}


def _build_nc():
    import concourse.tile as tile
    from concourse import bacc, mybir

    bf16 = mybir.dt.bfloat16
    f32 = mybir.dt.float32
    Exp = mybir.ActivationFunctionType.Exp

    nc = bacc.Bacc(None, target_bir_lowering=False)

    # head0: [wq_e0(256) | x_e0[:,0:512](512) | wq_e1(256) | x_e1[:,0:512](512)]
    head0 = nc.declare_dram_parameter("head0", [P, 2 * (2 * P + HC)], bf16,
                                      isOutput=False)
    # wkv: [wk pk (512) | wv pk (512)]
    wkv = nc.declare_dram_parameter("wkv", [P, 2 * EB * D], bf16, isOutput=False)
    # x_rest: NXCH chunks of [x_e0[:,c0:c0+512] | x_e1[:,c0:c0+512]] (1024 each)
    x_rest = nc.declare_dram_parameter("x_rest", [P, NXCH * EB * QT], bf16,
                                       isOutput=False)
    # out carries the unnormalized AV numerator plus the softmax denominator
    # (column D); the host performs the final divide.
    out = nc.declare_dram_parameter("out", [SQ, D + 1], f32, isOutput=True)

    with tile.TileContext(nc) as tc, ExitStack() as ctx:
        consts = ctx.enter_context(tc.tile_pool(name="consts", bufs=1))
        ps = ctx.enter_context(tc.tile_pool(name="ps", bufs=4, space="PSUM"))
        po = ctx.enter_context(tc.tile_pool(name="po", bufs=4, space="PSUM"))
        work = ctx.enter_context(tc.tile_pool(name="work", bufs=5))
        outp = ctx.enter_context(tc.tile_pool(name="outp", bufs=2))

        # ---- SBUF destination tiles -----------------------------------------
        head_sb = consts.tile([P, 2 * (2 * P + HC)], bf16)
        wkv_sb = consts.tile([P, 2 * EB * D], bf16)
        x_sb = consts.tile([P, NXCH, EB, QT], bf16)  # x cols [512:4096)
        kt_sb = consts.tile([P, DB, S], bf16)  # K^T [d, k]
        qt_sb = consts.tile([P, DB, SQ], bf16)  # Q^T [d, q]
        v_sb = consts.tile([P, KB, D + 1], bf16)  # V [k, d] + ones column
        warm_l = consts.tile([P, P], bf16)
        warm_r = consts.tile([P, QT], bf16)

        # memsets first on DVE (instant; DVE issues no DMAs)
        nc.vector.memset(warm_l, 0.0)
        nc.vector.memset(warm_r, 0.0)
        nc.vector.memset(v_sb[:, :, D : D + 1], 1.0)

        # ---- input DMA: 3 parallel engine rings, consumption order ----------
        HW = 2 * P + HC  # 768 cols per head piece
        nc.sync.dma_start(out=head_sb[:, :HW], in_=head0[:, :HW])
        nc.scalar.dma_start(out=head_sb[:, HW:], in_=head0[:, HW:])
        # wk before wv: the first K^T matmul comes ~0.2us before the first V
        nc.gpsimd.dma_start(out=wkv_sb[:, : EB * D], in_=wkv[:, : EB * D])
        nc.gpsimd.dma_start(out=wkv_sb[:, EB * D :], in_=wkv[:, EB * D :])

        def xsrc(i):  # chunk i (x cols [512*(i+1), 512*(i+2))) as [p, a, m]
            return x_rest[:, i * EB * QT : (i + 1) * EB * QT].rearrange(
                "p (a m) -> p a m", a=EB
            )

        # chunk 0 is needed first and would otherwise land marginally late:
        # split it across the sync and scalar rings right behind the head
        # pieces, then stream the rest in consumption order. The scalar ring
        # gets nothing further: its queue must be free for PSUM evictions by
        # the time the first projections retire (~2us after the issues).
        nc.sync.dma_start(out=x_sb[:, 0, :, 0 : QT // 2],
                          in_=xsrc(0)[:, :, 0 : QT // 2])
        nc.scalar.dma_start(out=x_sb[:, 0, :, QT // 2 :],
                            in_=xsrc(0)[:, :, QT // 2 :])
        for eng, i in [
            (nc.sync, 1),
            (nc.gpsimd, 2),
            (nc.gpsimd, 3),
            (nc.gpsimd, 4),
            (nc.sync, 5),
            (nc.gpsimd, 6),
        ]:
            eng.dma_start(out=x_sb[:, i], in_=xsrc(i))

        def wq(ea):
            return head_sb[:, ea * HW : ea * HW + 2 * P]

        def xs(ea, c0, w):
            """x^T slice [128, w] for e-block ea, columns [c0, c0+w)."""
            if c0 + w <= HC:
                return head_sb[:, ea * HW + 2 * P + c0 : ea * HW + 2 * P + c0 + w]
            ch = c0 // QT - 1
            return x_sb[:, ch, ea, c0 % QT : c0 % QT + w]

        wk_sb = wkv_sb[:, 0 : EB * D].rearrange("p (a d) -> p a d", a=EB)
        wv_sb = wkv_sb[:, EB * D :].rearrange("p (a d) -> p a d", a=EB)

        # ---- PE warmup: dummy matmuls while the first DMAs land, so HAM
        # un-throttles (1.2 -> 2.4 GHz) by the time the projections run.
        for _ in range(4):
            wp = ps.tile([P, QT], f32, name="wp", tag="pt")
            nc.tensor.matmul(wp, lhsT=warm_l, rhs=warm_r, start=True, stop=True)

        # ---- projection pieces ----------------------------------------------
        # PSUM eviction casts are split across DVE and ScalarE: either engine
        # alone is slower than PE through this phase.
        def qt_part(kc, da):
            sl = slice(kc * QT, (kc + 1) * QT)
            pt = ps.tile([P, QT], f32, name="pt", tag="pt")
            for ea in range(EB):
                nc.tensor.matmul(
                    pt,
                    lhsT=wq(ea)[:, da * P : (da + 1) * P],
                    rhs=xs(ea, kc * QT, QT),
                    start=(ea == 0),
                    stop=(ea == EB - 1),
                )
            if da == 1:
                nc.scalar.copy(out=qt_sb[:, da, sl], in_=pt)
            else:
                nc.vector.tensor_copy(out=qt_sb[:, da, sl], in_=pt)

        def kt_part(kc, da):
            sl = slice(kc * QT, (kc + 1) * QT)
            pt = ps.tile([P, QT], f32, name="pt", tag="pt")
            for ea in range(EB):
                nc.tensor.matmul(
                    pt,
                    lhsT=wk_sb[:, ea, da * P : (da + 1) * P],
                    rhs=xs(ea, kc * QT, QT),
                    start=(ea == 0),
                    stop=(ea == EB - 1),
                )
            if da == 1:
                nc.scalar.copy(out=kt_sb[:, da, sl], in_=pt)
            else:
                nc.vector.tensor_copy(out=kt_sb[:, da, sl], in_=pt)

        def v_part(kb):
            pt = ps.tile([P, QT], f32, name="pt", tag="pt")
            for ea in range(EB):
                nc.tensor.matmul(
                    pt[:, :D],
                    lhsT=xs(ea, kb * P, P),
                    rhs=wv_sb[:, ea, :],
                    start=(ea == 0),
                    stop=(ea == EB - 1),
                )
            if kb % 2 == 1:
                nc.scalar.copy(out=v_sb[:, kb, 0:D], in_=pt[:, :D])
            else:
                nc.vector.tensor_copy(out=v_sb[:, kb, 0:D], in_=pt[:, :D])

        def proj(kc):
            # sandwich every V matmul between 512-wide Q^T/K^T streams so
            # each V LDWEIGHTS prefetches fully under a long stream
            kb0 = kc * (QT // P)
            if kc * QT < SQ:
                qt_part(kc, 0)
                v_part(kb0)
                kt_part(kc, 0)
                v_part(kb0 + 1)
                qt_part(kc, 1)
                v_part(kb0 + 2)
                kt_part(kc, 1)
                v_part(kb0 + 3)
            else:
                kt_part(kc, 0)
                v_part(kb0)
                v_part(kb0 + 1)
                kt_part(kc, 1)
                v_part(kb0 + 2)
                v_part(kb0 + 3)

        # ---- attention ------------------------------------------------------
        inv_sqrt_d = 1.0 / np.sqrt(D)

        for kc in range(S // QT):
            proj(kc)

        for qb in range(NQB):
            last_qb = qb == NQB - 1
            po_tiles = [
                po.tile([P, D + 1], f32, name="po_acc", tag="po_acc")
                for _ in range(SUBQ)
            ]
            pend = []  # (attn_tile, kb) waiting for their AV matmuls

            def emit_av(at, kb):
                for sub in range(SUBQ):
                    nc.tensor.matmul(
                        po_tiles[sub],
                        lhsT=at[:, sub * P : (sub + 1) * P],
                        rhs=v_sb[:, kb, :],
                        start=(kb == 0),
                        stop=(kb == KB - 1),
                    )

            for kb in range(KB):
                pt = ps.tile([P, QT], f32)
                for da in range(DB):
                    nc.tensor.matmul(
                        pt,
                        lhsT=kt_sb[:, da, kb * P : (kb + 1) * P],
                        rhs=qt_sb[:, da, qb * QT : (qb + 1) * QT],
                        start=(da == 0),
                        stop=(da == DB - 1),
                    )
                if last_qb and kb == KB - 1:
                    # the final exp is on the kernel's critical path: drain
                    # the pipeline, then split it into four 128-wide pieces
                    # so each AV can start as soon as its quarter is ready.
                    for a, k in pend:
                        emit_av(a, k)
                    pend = []
                    at = work.tile([P, QT], bf16)
                    for sub in range(SUBQ):
                        qsl = slice(sub * P, (sub + 1) * P)
                        nc.scalar.activation(
                            out=at[:, qsl], in_=pt[:, qsl], func=Exp,
                            scale=inv_sqrt_d,
                        )
                        nc.tensor.matmul(
                            po_tiles[sub],
                            lhsT=at[:, qsl],
                            rhs=v_sb[:, kb, :],
                            start=False,
                            stop=True,
                        )
                    continue
                at = work.tile([P, QT], bf16)
                nc.scalar.activation(out=at, in_=pt, func=Exp, scale=inv_sqrt_d)
                # software-pipeline AV so exp(kb) has slack and AV
                # weight-loads never stall PE; drain eagerly near the end.
                depth = 2 if (last_qb and kb >= KB - 5) else 4
                pend.append((at, kb))
                while len(pend) > depth:
                    emit_av(*pend.pop(0))
            for a, k in pend:
                emit_av(a, k)

            # evict numerator+denominator f32 to SBUF (DVE/ACT split), then
            # DMA; the host performs the divide.
            ob = outp.tile([P, SUBQ, D + 1], f32)
            for sub in range(SUBQ):
                if sub % 2 == 1:
                    nc.scalar.copy(out=ob[:, sub, :], in_=po_tiles[sub][:, :])
                else:
                    nc.vector.tensor_copy(out=ob[:, sub, :], in_=po_tiles[sub][:, :])
            r0 = qb * QT
            if last_qb:
                # parallel engine rings to drain the tail fast
                engs = [nc.sync, nc.gpsimd, nc.scalar, nc.sync]
                for sub in range(SUBQ):
                    engs[sub].dma_start(
                        out=out[r0 + sub * P : r0 + (sub + 1) * P, :],
                        in_=ob[:, sub, :],
                    )
            else:
                eng = [nc.sync, nc.gpsimd, nc.sync][qb]
                eng.dma_start(
                    out=out[r0 : r0 + QT, :].rearrange("(s p) e -> p s e", p=P),
                    in_=ob,
                )

    nc.finalize()
    return nc


def _ensure_ntff_hook():
    """This image's antenv lacks axon_hooks; synthesize it from the ctypes
    implementation in trn_agent_boot so trace=True can capture NTFF profiles."""
    import types

    try:
        from antenv.axon_hooks import get_axon_ntff_profile_hook  # noqa: F401

        return
    except ImportError:
        pass
    import antenv  # noqa: F401
    from trn_agent_boot.trn_boot import _ntff_profile_via_ctypes

    hook = _ntff_profile_via_ctypes("/opt/axon/libaxon_pjrt.so")
    mod = types.ModuleType("antenv.axon_hooks")
    mod.get_axon_ntff_profile_hook = lambda: hook
    mod.set_axon_ntff_profile_hook = lambda h: None
    sys.modules["antenv.axon_hooks"] = mod


def kernel(x, Wq, Wk, Wv):
    from concourse.bass_utils import run_bass_kernel_spmd

    global LAST_RESULT
    if "nc" not in _CACHE:
        _CACHE["nc"] = _build_nc()
    nc = _CACHE["nc"]

    bf = ml_dtypes.bfloat16
    x = np.asarray(x, dtype=np.float32)
    xT = np.ascontiguousarray(x.transpose(0, 2, 1)).astype(bf)  # [B, D, S]
    wqt = np.asarray(Wq, np.float32).T.astype(bf)
    wkt = np.asarray(Wk, np.float32).T.astype(bf)
    wvt = np.asarray(Wv, np.float32).T.astype(bf)

    def pk(a2d):  # [256, w] -> [128, 2*w] (e-blocks adjacent per partition)
        w = a2d.shape[1]
        return a2d.reshape(2, P, w).transpose(1, 0, 2).reshape(P, 2 * w)

    wq_e = wqt.reshape(2, P, D)  # [ea][p][d]
    wkv_np = np.ascontiguousarray(np.concatenate([pk(wkt), pk(wvt)], axis=1))

    in_maps = []
    for c in range(NCORES):
        b, qc = c // 2, c % 2
        if qc == 0:
            xr = xT[b]
        else:
            # rotate so this core's query half occupies columns [0:SQ);
            # key order is irrelevant to softmax attention.
            xr = np.concatenate([xT[b][:, SQ:], xT[b][:, :SQ]], axis=1)
        xe = xr.reshape(2, P, S)  # [ea][p][col]
        head = np.ascontiguousarray(
            np.concatenate(
                [wq_e[0], xe[0][:, 0:HC], wq_e[1], xe[1][:, 0:HC]], axis=1
            )
        )
        rest = np.ascontiguousarray(
            np.concatenate(
                [
                    np.concatenate(
                        [xe[0][:, c0 : c0 + QT], xe[1][:, c0 : c0 + QT]], axis=1
                    )
                    for c0 in range(HC, S, QT)
                ],
                axis=1,
            )
        )
        in_maps.append({"head0": head, "wkv": wkv_np, "x_rest": rest})

    trace = bool(int(os.environ.get("KERNEL_TRACE", "0")))
    if trace:
        _ensure_ntff_hook()
    LAST_RESULT = run_bass_kernel_spmd(
        nc, in_maps, core_ids=list(range(NCORES)), trace=trace
    )
    outs = [LAST_RESULT.results[c]["out"] for c in range(NCORES)]
    full = np.empty((B, S, D), dtype=np.float32)
    for c in range(NCORES):
        b, qc = c // 2, c % 2
        o = outs[c]
        full[b, qc * SQ : (qc + 1) * SQ, :] = o[:, :D] / o[:, D : D + 1]
    return full


# revision 14
# speedup vs baseline: 1.0319x; 1.0319x over previous
"""Distributed single-head attention block for one TRN2 chip (8 NeuronCores).

Math (per batch b):  Q = x@Wq.T, K = x@Wk.T, V = x@Wv.T,
                     out = softmax(Q K^T / sqrt(D)) V
Shapes: x [4, 4096, 256], W* [256, 256], out [4, 4096, 256] (f32).

Sharding: core c handles batch b = c//2, query half qc = c%2 (2048 queries),
with full K/V for that batch (K/V projection recomputed on both cores of a
batch pair -- it is tiny). All matmul inputs are pre-transposed & bf16-cast on
the host so that no on-chip transposes are needed.

Attention is permutation-invariant over keys, so each core receives x^T
ROTATED so that its own query half occupies columns [0:2048] -- Q projects
straight from the head of the same buffer K/V project from.

Schedule (v5):
  - input DMA split across 3 engine rings (sync/scalar/gpsimd) with the two
    head pieces sized to the minimum the first Q^T accumulation needs, and
    the x tail in 128-256KB chunks issued in consumption order.
  - scores are computed *transposed* (tiles [k=128, q=512]): PE matmul with
    lhsT = K^T tile, rhs = Q^T tile; exp runs on ScalarE straight out of
    PSUM (scale=1/16 folded in, no max subtraction: |scores| <= ~11).
  - attn^T tiles feed the AV matmul as the stationary operand with V [k, d]
    as moving; a ones column appended to V makes the same PSUM accumulation
    produce the softmax denominator.
  - the kernel returns the unnormalized numerator + denominator (D+1 f32
    columns); the final divide happens on the host. This removes all
    reciprocal/scale work from the device and shortens the tail: the PSUM
    accumulators are simply evicted f32 -> SBUF (DVE/ACT split) and DMAd.
  - the final q-tile drains its AV pipeline eagerly, splits the last exp
    into four 128-wide pieces (pipelining the last AVs behind them), and
    scatters its four sub-tiles over three engine rings.
"""

import os
import sys
from contextlib import ExitStack

sys.path.insert(0, "/opt/trn_rl_repo")

import numpy as np
import ml_dtypes

B, S, D = 4, 4096, 256
NCORES = 8
SQ = S // 2  # queries per core
P = 128  # SBUF partitions
EB = D // P  # e (contraction) blocks for projections
DB = D // P  # d blocks
KB = S // P  # key blocks of 128
QT = 512  # q tile (matmul moving free dim)
NQB = SQ // QT  # q tiles per core
SUBQ = QT // P  # 128-query sub-blocks per q tile
HC = 512  # head chunk: x columns packed with wq
NXCH = (S - HC) // QT  # 7 tail chunks of 512 columns

LAST_RESULT = None  # BassKernelResults of the most recent run (for test.py)
_CACHE = {}


def _build_nc():
    import concourse.tile as tile
    from concourse import bacc, mybir

    bf16 = mybir.dt.bfloat16
    f32 = mybir.dt.float32
    Exp = mybir.ActivationFunctionType.Exp

    nc = bacc.Bacc(None, target_bir_lowering=False)

    # head0: [wq pk (512) | x[:,0:512] pk (1024)] -- one sync-ring DMA sized so
    # the PE warmup bridges its arrival with no gap (a PE gap >~0.5us during
    # the HAM ramp resets the clock ramp and costs far more than it saves).
    head0 = nc.declare_dram_parameter("head0", [P, EB * D + EB * HC], bf16,
                                      isOutput=False)
    # wkv: [wk pk (512) | wv pk (512)]
    wkv = nc.declare_dram_parameter("wkv", [P, 2 * EB * D], bf16, isOutput=False)
    # x_rest: NXCH chunks of [x_e0[:,c0:c0+512] | x_e1[:,c0:c0+512]] (1024 each)
    x_rest = nc.declare_dram_parameter("x_rest", [P, NXCH * EB * QT], bf16,
                                       isOutput=False)
    # out carries the unnormalized AV numerator plus the softmax denominator
    # (column D); the host performs the final divide.
    out = nc.declare_dram_parameter("out", [SQ, D + 1], f32, isOutput=True)

    with tile.TileContext(nc) as tc, ExitStack() as ctx:
        consts = ctx.enter_context(tc.tile_pool(name="consts", bufs=1))
        ps = ctx.enter_context(tc.tile_pool(name="ps", bufs=4, space="PSUM"))
        po = ctx.enter_context(tc.tile_pool(name="po", bufs=4, space="PSUM"))
        work = ctx.enter_context(tc.tile_pool(name="work", bufs=5))
        outp = ctx.enter_context(tc.tile_pool(name="outp", bufs=2))

        # ---- SBUF destination tiles -----------------------------------------
        head_sb = consts.tile([P, EB * D + EB * HC], bf16)
        wkv_sb = consts.tile([P, 2 * EB * D], bf16)
        x_sb = consts.tile([P, NXCH, EB, QT], bf16)  # x cols [512:4096)
        kt_sb = consts.tile([P, DB, S], bf16)  # K^T [d, k]
        qt_sb = consts.tile([P, DB, SQ], bf16)  # Q^T [d, q]
        v_sb = consts.tile([P, KB, D + 1], bf16)  # V [k, d] + ones column
        warm_l = consts.tile([P, P], bf16)
        warm_r = consts.tile([P, QT], bf16)

        # memsets first on DVE (instant; DVE issues no DMAs)
        nc.vector.memset(warm_l, 0.0)
        nc.vector.memset(warm_r, 0.0)
        nc.vector.memset(v_sb[:, :, D : D + 1], 1.0)

        # ---- input DMA: 3 parallel engine rings, consumption order ----------
        # sync carries exactly what the first Q^T matmuls need; scalar carries
        # wk|wv (no further scalar issues -- its queue must be free for PSUM
        # evictions); gpsimd streams the early x chunks.
        nc.sync.dma_start(out=head_sb, in_=head0[:, :])
        nc.scalar.dma_start(out=wkv_sb, in_=wkv[:, :])

        def xsrc(i):  # chunk i (x cols [512*(i+1), 512*(i+2))) as [p, a, m]
            return x_rest[:, i * EB * QT : (i + 1) * EB * QT].rearrange(
                "p (a m) -> p a m", a=EB
            )

        for eng, i in [
            (nc.gpsimd, 0),
            (nc.sync, 1),
            (nc.gpsimd, 2),
            (nc.gpsimd, 3),
            (nc.scalar, 4),
            (nc.sync, 5),
            (nc.gpsimd, 6),
        ]:
            eng.dma_start(out=x_sb[:, i], in_=xsrc(i))

        wq_sb = head_sb[:, 0 : EB * D].rearrange("p (a d) -> p a d", a=EB)
        x_head = head_sb[:, EB * D :].rearrange("p (a m) -> p a m", a=EB)

        def wq(ea):
            return wq_sb[:, ea, :]

        def xs(ea, c0, w):
            """x^T slice [128, w] for e-block ea, columns [c0, c0+w)."""
            if c0 + w <= HC:
                return x_head[:, ea, c0 : c0 + w]
            ch = c0 // QT - 1
            return x_sb[:, ch, ea, c0 % QT : c0 % QT + w]

        wk_sb = wkv_sb[:, 0 : EB * D].rearrange("p (a d) -> p a d", a=EB)
        wv_sb = wkv_sb[:, EB * D :].rearrange("p (a d) -> p a d", a=EB)

        # ---- PE warmup: dummy matmuls while the first DMAs land, so HAM
        # un-throttles (1.2 -> 2.4 GHz) by the time the projections run, and
        # the PE never idles long enough to reset the ramp.
        for _ in range(6):
            wp = ps.tile([P, QT], f32, name="wp", tag="pt")
            nc.tensor.matmul(wp, lhsT=warm_l, rhs=warm_r, start=True, stop=True)

        # ---- projection pieces ----------------------------------------------
        # PSUM eviction casts are split across DVE and ScalarE: either engine
        # alone is slower than PE through this phase.
        def qt_part(kc, da):
            sl = slice(kc * QT, (kc + 1) * QT)
            pt = ps.tile([P, QT], f32, name="pt", tag="pt")
            for ea in range(EB):
                nc.tensor.matmul(
                    pt,
                    lhsT=wq(ea)[:, da * P : (da + 1) * P],
                    rhs=xs(ea, kc * QT, QT),
                    start=(ea == 0),
                    stop=(ea == EB - 1),
                )
            if da == 1:
                nc.scalar.copy(out=qt_sb[:, da, sl], in_=pt)
            else:
                nc.vector.tensor_copy(out=qt_sb[:, da, sl], in_=pt)

        def kt_part(kc, da):
            sl = slice(kc * QT, (kc + 1) * QT)
            pt = ps.tile([P, QT], f32, name="pt", tag="pt")
            for ea in range(EB):
                nc.tensor.matmul(
                    pt,
                    lhsT=wk_sb[:, ea, da * P : (da + 1) * P],
                    rhs=xs(ea, kc * QT, QT),
                    start=(ea == 0),
                    stop=(ea == EB - 1),
                )
            if da == 1:
                nc.scalar.copy(out=kt_sb[:, da, sl], in_=pt)
            else:
                nc.vector.tensor_copy(out=kt_sb[:, da, sl], in_=pt)

        def v_part(kb):
            pt = ps.tile([P, QT], f32, name="pt", tag="pt")
            for ea in range(EB):
                nc.tensor.matmul(
                    pt[:, :D],
                    lhsT=xs(ea, kb * P, P),
                    rhs=wv_sb[:, ea, :],
                    start=(ea == 0),
                    stop=(ea == EB - 1),
                )
            if kb % 2 == 1:
                nc.scalar.copy(out=v_sb[:, kb, 0:D], in_=pt[:, :D])
            else:
                nc.vector.tensor_copy(out=v_sb[:, kb, 0:D], in_=pt[:, :D])

        def proj(kc):
            # sandwich every V matmul between 512-wide Q^T/K^T streams so
            # each V LDWEIGHTS prefetches fully under a long stream
            kb0 = kc * (QT // P)
            if kc * QT < SQ:
                qt_part(kc, 0)
                v_part(kb0)
                kt_part(kc, 0)
                v_part(kb0 + 1)
                qt_part(kc, 1)
                v_part(kb0 + 2)
                kt_part(kc, 1)
                v_part(kb0 + 3)
            else:
                kt_part(kc, 0)
                v_part(kb0)
                v_part(kb0 + 1)
                kt_part(kc, 1)
                v_part(kb0 + 2)
                v_part(kb0 + 3)

        # ---- attention ------------------------------------------------------
        inv_sqrt_d = 1.0 / np.sqrt(D)

        for kc in range(S // QT):
            proj(kc)

        for qb in range(NQB):
            last_qb = qb == NQB - 1
            po_tiles = [
                po.tile([P, D + 1], f32, name="po_acc", tag="po_acc")
                for _ in range(SUBQ)
            ]
            pend = []  # (attn_tile, kb) waiting for their AV matmuls

            def emit_av(at, kb):
                for sub in range(SUBQ):
                    nc.tensor.matmul(
                        po_tiles[sub],
                        lhsT=at[:, sub * P : (sub + 1) * P],
                        rhs=v_sb[:, kb, :],
                        start=(kb == 0),
                        stop=(kb == KB - 1),
                    )

            for kb in range(KB):
                pt = ps.tile([P, QT], f32)
                for da in range(DB):
                    nc.tensor.matmul(
                        pt,
                        lhsT=kt_sb[:, da, kb * P : (kb + 1) * P],
                        rhs=qt_sb[:, da, qb * QT : (qb + 1) * QT],
                        start=(da == 0),
                        stop=(da == DB - 1),
                    )
                if last_qb and kb == KB - 1:
                    # the final exp is on the kernel's critical path: drain
                    # the pipeline, then split it into four 128-wide pieces
                    # so each AV can start as soon as its quarter is ready.
                    for a, k in pend:
                        emit_av(a, k)
                    pend = []
                    at = work.tile([P, QT], bf16)
                    for sub in range(SUBQ):
                        qsl = slice(sub * P, (sub + 1) * P)
                        nc.scalar.activation(
                            out=at[:, qsl], in_=pt[:, qsl], func=Exp,
                            scale=inv_sqrt_d,
                        )
                        nc.tensor.matmul(
                            po_tiles[sub],
                            lhsT=at[:, qsl],
                            rhs=v_sb[:, kb, :],
                            start=False,
                            stop=True,
                        )
                    continue
                at = work.tile([P, QT], bf16)
                nc.scalar.activation(out=at, in_=pt, func=Exp, scale=inv_sqrt_d)
                # software-pipeline AV so exp(kb) has slack and AV
                # weight-loads never stall PE; drain eagerly near the end.
                depth = 2 if (last_qb and kb >= KB - 5) else 4
                pend.append((at, kb))
                while len(pend) > depth:
                    emit_av(*pend.pop(0))
            for a, k in pend:
                emit_av(a, k)

            # evict numerator+denominator f32 to SBUF (DVE/ACT split), then
            # DMA; the host performs the divide.
            ob = outp.tile([P, SUBQ, D + 1], f32)
            for sub in range(SUBQ):
                if sub % 2 == 1:
                    nc.scalar.copy(out=ob[:, sub, :], in_=po_tiles[sub][:, :])
                else:
                    nc.vector.tensor_copy(out=ob[:, sub, :], in_=po_tiles[sub][:, :])
            r0 = qb * QT
            if last_qb:
                # parallel engine rings to drain the tail fast
                engs = [nc.sync, nc.gpsimd, nc.scalar, nc.sync]
                for sub in range(SUBQ):
                    engs[sub].dma_start(
                        out=out[r0 + sub * P : r0 + (sub + 1) * P, :],
                        in_=ob[:, sub, :],
                    )
            else:
                eng = [nc.sync, nc.gpsimd, nc.sync][qb]
                eng.dma_start(
                    out=out[r0 : r0 + QT, :].rearrange("(s p) e -> p s e", p=P),
                    in_=ob,
                )

    nc.finalize()
    return nc


def _ensure_ntff_hook():
    """This image's antenv lacks axon_hooks; synthesize it from the ctypes
    implementation in trn_agent_boot so trace=True can capture NTFF profiles."""
    import types

    try:
        from antenv.axon_hooks import get_axon_ntff_profile_hook  # noqa: F401

        return
    except ImportError:
        pass
    import antenv  # noqa: F401
    from trn_agent_boot.trn_boot import _ntff_profile_via_ctypes

    hook = _ntff_profile_via_ctypes("/opt/axon/libaxon_pjrt.so")
    mod = types.ModuleType("antenv.axon_hooks")
    mod.get_axon_ntff_profile_hook = lambda: hook
    mod.set_axon_ntff_profile_hook = lambda h: None
    sys.modules["antenv.axon_hooks"] = mod


def kernel(x, Wq, Wk, Wv):
    from concourse.bass_utils import run_bass_kernel_spmd

    global LAST_RESULT
    if "nc" not in _CACHE:
        _CACHE["nc"] = _build_nc()
    nc = _CACHE["nc"]

    bf = ml_dtypes.bfloat16
    x = np.asarray(x, dtype=np.float32)
    xT = np.ascontiguousarray(x.transpose(0, 2, 1)).astype(bf)  # [B, D, S]
    wqt = np.asarray(Wq, np.float32).T.astype(bf)
    wkt = np.asarray(Wk, np.float32).T.astype(bf)
    wvt = np.asarray(Wv, np.float32).T.astype(bf)

    def pk(a2d):  # [256, w] -> [128, 2*w] (e-blocks adjacent per partition)
        w = a2d.shape[1]
        return a2d.reshape(2, P, w).transpose(1, 0, 2).reshape(P, 2 * w)

    wkv_np = np.ascontiguousarray(np.concatenate([pk(wkt), pk(wvt)], axis=1))

    in_maps = []
    for c in range(NCORES):
        b, qc = c // 2, c % 2
        if qc == 0:
            xr = xT[b]
        else:
            # rotate so this core's query half occupies columns [0:SQ);
            # key order is irrelevant to softmax attention.
            xr = np.concatenate([xT[b][:, SQ:], xT[b][:, :SQ]], axis=1)
        xe = xr.reshape(2, P, S)  # [ea][p][col]
        head = np.ascontiguousarray(
            np.concatenate([pk(wqt), pk(xr[:, 0:HC])], axis=1)
        )
        rest = np.ascontiguousarray(
            np.concatenate(
                [
                    np.concatenate(
                        [xe[0][:, c0 : c0 + QT], xe[1][:, c0 : c0 + QT]], axis=1
                    )
                    for c0 in range(HC, S, QT)
                ],
                axis=1,
            )
        )
        in_maps.append({"head0": head, "wkv": wkv_np, "x_rest": rest})

    trace = bool(int(os.environ.get("KERNEL_TRACE", "0")))
    if trace:
        _ensure_ntff_hook()
    LAST_RESULT = run_bass_kernel_spmd(
        nc, in_maps, core_ids=list(range(NCORES)), trace=trace
    )
    outs = [LAST_RESULT.results[c]["out"] for c in range(NCORES)]
    full = np.empty((B, S, D), dtype=np.float32)
    for c in range(NCORES):
        b, qc = c // 2, c % 2
        o = outs[c]
        full[b, qc * SQ : (qc + 1) * SQ, :] = o[:, :D] / o[:, D : D + 1]
    return full
